# revision 32
# baseline (speedup 1.0000x reference)
"""Distributed causal self-attention for TRN2 (8 NeuronCores).

Sharding: tensor-parallel over heads (2 heads/core). Each core computes
q,k,v for its heads over the full sequence (column-sharded c_attn), runs
causal attention for them (chunk order 1,2,3,0 so the last chunk is the
shortest), reshards the attention output sequence-wise with two grouped
AllToAlls ((1,2) then (3,0)), and applies the full output projection to
its 256 rows (row-sharded c_proj). A tiny dummy AllToAll at kernel start
hoists the NRT entry barrier + first-collective ncfw setup into the
compute prologue. The AV matmul's lhsT is [v_h | ones-block] (M=128):
the exp-sums come out replicated across 64 psum rows for free, so the
softmax normalize is copy+reciprocal+mul on the vector engine.

Round-1 optimizations over the original baseline:
  - wide-line input loads: xt as 8 [128, 2048] tiles (4KB lines, 2 HWDGE
    queues), wqkv as ONE [128, 3072] tile (6KB lines, host pre-swizzled)
    -> the prologue valley (PE starved until ~24us) shrinks to ~13us.
  - wpt load gated on the first qkv output so its 2MB doesn't steal
    prologue HBM bandwidth from xt.
  - chunk-0 (the last chunk) runs its AVs in a pslog-pool tile and its
    logits through psmm tiles, breaking the psav WAR on chunk-3's
    normalize (-2.7us), and skips the sm staging copy in its normalize.
  - gathered a2a loads batched (1 DMA for at12, 2 for at30) on queues
    that are idle at that point (sync / scalar+gpsimd).
  - a scalar-engine copy chain (ACT is idle then) gates 4 tiny matmuls
    spread across the final-collective wait so the HAM clock gate never
    sees an idle window -> proj2 runs at 2.4GHz instead of 1.2.
  - proj writes go straight from PSUM to DRAM (no ob staging copy).

Row ownership is striped: within q-chunk qc (512 rows), rows
[512*qc + 64*j : 512*qc + 64*(j+1)] belong to core j. Core j's "out"
holds its 4 stripes in qc order 1,2,3,0; the host reassembles.

Compute dtype: bf16 operands, fp32 PSUM accumulation.
"""

import numpy as np
import ml_dtypes

import concourse.bass as bass
import concourse.mybir as mybir
import concourse.tile as tile
from concourse import bacc
from concourse.masks import make_identity, make_upper_triangular
from concourse.tile import add_dep_helper

S, E, H = 2048, 1024, 16
D = E // H          # 64 head dim
NCORES = 8
HPC = H // NCORES   # 2 heads per core
F = HPC * D         # 128 local features
SQ = S // NCORES    # 256 output rows per core
ST = 64             # per-core stripe within a q chunk
P = 128
QC = 512            # q chunk (columns per attention pass)
NQC = S // QC       # 4
NKB = S // P        # 16 k blocks
KCH = E // P        # 8 contraction chunks for E-dim matmuls

F32 = mybir.dt.float32
BF16 = mybir.dt.bfloat16
FP8 = mybir.dt.float8e4
EXP = mybir.ActivationFunctionType.Exp
DR = mybir.MatmulPerfMode.DoubleRow
# fp8 pre-scales (folded out by the 1/(SX*SW) factor in the qkv bias add)
SX = 4.0
SW = 64.0

# attention chunk order: 1,2,3,0 (last chunk is the cheapest so the final
# a2a triggers as early as possible). a2a groups: [1,2] merged, then [3,0].
AORDER = [1, 2, 3, 0]


def build_nc():
    nc = bacc.Bacc("TRN2", target_bir_lowering=False, debug=False,
                   num_devices=NCORES, enable_partition_id=True)

    xt = nc.dram_tensor("xt", [E, S], FP8, kind="ExternalInput")
    wqkv = nc.dram_tensor("wqkv", [P, 3 * KCH * P], FP8, kind="ExternalInput")
    bqkv = nc.dram_tensor("bqkv", [P, 3], F32, kind="ExternalInput")
    wpt = nc.dram_tensor("wpt", [E, E], BF16, kind="ExternalInput")
    bp = nc.dram_tensor("bp", [1, E], BF16, kind="ExternalInput")
    out = nc.dram_tensor("out", [SQ, E], F32, kind="ExternalOutput")

    with tile.TileContext(nc) as tc:
        _body(nc, tc, xt, wqkv, bqkv, wpt, bp, out)

    nc.compile()
    return nc


def _body(nc, tc, xt, wqkv, bqkv, wpt, bp, out):
    import contextlib
    ctx = contextlib.ExitStack()
    with ctx:
        constp = ctx.enter_context(tc.tile_pool(name="constp", bufs=1))
        wqp = ctx.enter_context(tc.tile_pool(name="wqp", bufs=1))
        xtp = ctx.enter_context(tc.tile_pool(name="xtp", bufs=1))
        qkvp = ctx.enter_context(tc.tile_pool(name="qkvp", bufs=1))
        vop = ctx.enter_context(tc.tile_pool(name="vop", bufs=1))
        wptp = ctx.enter_context(tc.tile_pool(name="wptp", bufs=1))
        atp = ctx.enter_context(tc.tile_pool(name="atp", bufs=1))
        expp = ctx.enter_context(tc.tile_pool(name="expp", bufs=7))
        stagep = ctx.enter_context(tc.tile_pool(name="stagep", bufs=3))
        smallp = ctx.enter_context(tc.tile_pool(name="smallp", bufs=4))
        warmp = ctx.enter_context(tc.tile_pool(name="warmp", bufs=1))
        outp = ctx.enter_context(tc.tile_pool(name="outp", bufs=2))
        psmm = ctx.enter_context(tc.tile_pool(name="psmm", bufs=2, space="PSUM"))
        pslog = ctx.enter_context(tc.tile_pool(name="pslog", bufs=2, space="PSUM"))
        psav = ctx.enter_context(tc.tile_pool(name="psav", bufs=1, space="PSUM"))
        dramp = ctx.enter_context(tc.tile_pool(name="dramp", bufs=1, space="DRAM"))

        # ---- dummy collective: fire a 1KB AllToAll immediately so the NRT
        # entry barrier + first-collective ncfw setup run during the compute
        # prologue instead of delaying the first real reshard op.
        dum_sb = constp.tile([NCORES, ST], BF16, name="dum_sb")
        nc.vector.memset(dum_sb[:, :], 0.0)
        dum_in = dramp.tile([NCORES, ST], BF16, name="dum_in", tag="dumi")
        dum_out = dramp.tile([NCORES, ST], BF16, name="dum_out", tag="dumo")
        nc.sync.dma_start(dum_in[:, :], dum_sb[:, :])
        nc.gpsimd.collective_compute(
            "AllToAll", mybir.AluOpType.bypass,
            replica_groups=[list(range(NCORES))],
            ins=[dum_in[:, :].opt()], outs=[dum_out[:, :].opt()])

        # ---- input loads first: fp8 xt as 4 row-pair tiles (DoubleRow
        # layout [p, j, s], 2KB lines) split across the two HWDGE queues;
        # fp8 wqkv as one pre-swizzled tile so K/V/Q weights land fast.
        wq_all = wqp.tile([P, 3 * KCH * P], FP8, name="wq_all")
        nc.scalar.dma_start(wq_all[:, :], wqkv[:, :])

        def wq_sl(m, g):
            # DoubleRow stationary slice [p, j, c] for contraction pair g
            base = (m * (KCH // 2) + g) * 2 * P
            return wq_all[:, base:base + 2 * P].rearrange(
                "p (j c) -> p j c", j=2)

        xt_sb = [xtp.tile([P, 2, S], FP8, name=f"xt_sb{g}", tag=f"xt{g}")
                 for g in range(KCH // 2)]
        for g, eng in enumerate((nc.sync, nc.scalar, nc.gpsimd, nc.sync)):
            eng.dma_start(
                xt_sb[g][:, :, :],
                xt[2 * g * P:(2 * g + 2) * P, :]
                .rearrange("(j p) s -> p j s", j=2))

        bq_sb = constp.tile([P, 3], F32, name="bq_sb")
        nc.sync.dma_start(bq_sb[:, :], bqkv[:, :])

        # ---- constants (built in f32, cast-copied to bf16) --------------
        warm = constp.tile([P, QC], BF16, name="warm")
        nc.vector.memset(warm[:, :], 0.0)
        ident_f = constp.tile([P, P], F32, name="ident_f")
        make_identity(nc, ident_f[:, :])
        ident = constp.tile([P, P], BF16, name="ident")
        nc.vector.tensor_copy(ident[:, :], ident_f[:, :])
        tri_f = constp.tile([P, P], F32, name="tri_f")  # tri[k,q] = 1 if q >= k
        make_upper_triangular(nc, tri_f[:, :], val=1.0, diag=True)
        tri = constp.tile([P, P], BF16, name="tri")
        nc.vector.tensor_copy(tri[:, :], tri_f[:, :])
        ones_f = constp.tile([P, 1], F32, name="ones_f")
        nc.vector.memset(ones_f[:, :], 1.0)
        ones1 = constp.tile([1, P], BF16, name="ones1")
        nc.vector.tensor_copy(ones1[:, :], ones_f[0:1, 0:1].to_broadcast((1, P)))

        # dependency-free warm-up matmuls: keep the PE busy through the HAM
        # activity window while the input DMAs are in flight, so the real
        # matmuls start at 2.4 GHz instead of 1.2
        for _ in range(8):
            wp_ps = psmm.tile([P, QC], F32, tag="mmp", name="warm_ps")
            nc.tensor.matmul(wp_ps[:, :], lhsT=warm[:, 0:P], rhs=warm[:, :],
                             start=True, stop=True)

        # separate q/k/v tiles per chunk so consumers only wait on the
        # piece they read (whole-tile deps otherwise delay attention start)
        qkv_sb = [[qkvp.tile([P, QC], BF16, name=f"qkv_sb{n}_{m}",
                             tag=f"qkv{n}_{m}") for m in range(3)]
                  for n in range(NQC)]
        # per-head lhsT layout [v_h (64) | ones (64)]: the AV matmul costs N
        # cycles regardless of M, so widening M from 65 to 128 replicates
        # the exp-sums across 64 psum rows for free.
        vones = [vop.tile([P, 4 * D], BF16, name=f"vones{kb}",
                          tag=f"vo{kb}") for kb in range(NKB)]
        for kb in range(NKB):
            nc.vector.memset(vones[kb][:, D:2 * D], 1.0)
            nc.vector.memset(vones[kb][:, 3 * D:4 * D], 1.0)

        # a2a groups: g0 = chunks (1,2), g1 = chunks (3,0). The CC stream
        # serializes ops AND gates each doorbell on the previous op's
        # completion, so exactly one op may remain once attention ends.
        GW = [2, 2]             # stripes per group
        GSLOT = {1: (0, 0), 2: (0, 1), 3: (1, 0), 0: (1, 1)}
        a2a_in = [dramp.tile([NCORES * F, GW[g] * ST], BF16,
                             name=f"a2a_in{g}", tag=f"ai{g}")
                  for g in range(2)]
        a2a_out = [dramp.tile([NCORES * F, GW[g] * ST], BF16,
                              name=f"a2a_out{g}", tag=f"ao{g}")
                   for g in range(2)]
        # gathered attention rows: one batched load for pair (1,2), two for
        # pair (3,0). at tile [P, k, 128]: contraction chunk k = sender k's
        # feature block; columns = the two 64-row stripes side by side.
        at12 = atp.tile([P, KCH, 2 * ST], BF16, name="at12")
        at30 = [atp.tile([P, KCH // 4, 2 * ST], BF16, name=f"at30_{i}")
                for i in range(4)]
        wp_sb = wptp.tile([P, KCH, E], BF16, name="wp_sb")
        bp_sb = constp.tile([1, E], BF16, name="bp_sb")
        # keep-warm chain tiles
        wa = [warmp.tile([P, 2048], BF16, name=f"wa{i}") for i in range(2)]
        nc.vector.memset(wa[0][:, :], 0.0)
        nc.vector.memset(wa[1][:, :], 0.0)

        def emit_qkv_m(n, m):
            pt = psmm.tile([P, QC], F32, tag="mmp", name="qkv_ps")
            for g in range(KCH // 2):
                nc.tensor.matmul(
                    pt[:, :], lhsT=wq_sl(m, g),
                    rhs=xt_sb[g][:, :, n * QC:(n + 1) * QC],
                    start=(g == 0), stop=(g == KCH // 2 - 1),
                    perf_mode=DR)
            # psum holds SX*SW * (w @ x); rescale and add the bias
            return nc.vector.scalar_tensor_tensor(
                qkv_sb[n][m][:, :], pt[:, :], 1.0 / (SX * SW),
                bq_sb[:, m:m + 1].to_broadcast((P, QC)),
                mybir.AluOpType.mult, mybir.AluOpType.add)

        def emit_vtrans(kb):
            n = kb // 4
            tp = psmm.tile([P, QC], BF16, tag="mmp", name="vt_ps")
            nc.tensor.transpose(
                tp[:, :P], qkv_sb[n][2][:, (kb % 4) * P:(kb % 4 + 1) * P],
                ident[:, :])
            vo = vones[kb]
            nc.vector.tensor_copy(vo[:, 0:D], tp[:, 0:D])
            nc.vector.tensor_copy(vo[:, 2 * D:3 * D], tp[:, D:2 * D])

        def emit_attn(qc, fillers=(), last=False, carry=(), defer=False):
            # fillers: emission callables sprinkled between k blocks so the
            # PE keeps independent work queued while exp stalls attention.
            # carry: deferred AV flushes + normalize of the PREVIOUS chunk,
            # drained during this chunk's first k blocks so the exp stream
            # never stalls at a chunk boundary.
            # last=True (chunk 0): AVs into a pslog tile + logits through
            # psmm so nothing WARs against chunk-3's psav normalize.
            fillers = list(fillers)
            carry = list(carry)
            nkb = 4 * qc + 4
            if last:
                avt = pslog.tile([P, 2 * QC], F32, tag="logp", name="av_last")

                def avsl(h, qoff):
                    return avt[:, h * QC + qoff:(h + 1) * QC]
            else:
                avp = [psav.tile([P, QC], F32, tag=f"avp{h}",
                                 name=f"av_ps{h}") for h in range(HPC)]

                def avsl(h, qoff):
                    return avp[h][:, qoff:QC]
            pend = []  # deferred attn@v (2-3 k blocks deep)

            def flush(item, is_last_kb):
                kb, et, qoff, N = item
                mm = None
                for h in range(HPC):
                    mm = nc.tensor.matmul(
                        avsl(h, qoff),
                        lhsT=vones[kb][:, 2 * D * h:2 * D * (h + 1)],
                        rhs=et[:, h, :N],
                        start=(kb == 0), stop=is_last_kb)
                return mm

            last_exp = None
            for kb in range(nkb):
                diag = kb >= 4 * qc
                qoff = P * (kb - 4 * qc) if diag else 0
                N = QC - qoff
                lqsl = slice(qoff, QC)
                et = expp.tile([P, 2, QC], BF16, tag="et", name="exp_sb")
                if last:
                    for h in range(HPC):
                        lp = psmm.tile([P, QC], F32, tag="mmp", name="log_ps")
                        nc.tensor.matmul(
                            lp[:, :N],
                            lhsT=qkv_sb[kb // 4][1][D * h:D * (h + 1),
                                                    (kb % 4) * P:(kb % 4 + 1) * P],
                            rhs=qkv_sb[qc][0][D * h:D * (h + 1), lqsl],
                            start=True, stop=True)
                        last_exp = nc.scalar.activation(
                            et[:, h, :N], lp[:, :N], EXP)
                else:
                    # two heads' logits into the two banks of one psum tile
                    lp = pslog.tile([P, 2 * QC], F32, tag="logp", name="log_ps")
                    for h in range(HPC):
                        nc.tensor.matmul(
                            lp[:, h * QC:h * QC + N],
                            lhsT=qkv_sb[kb // 4][1][D * h:D * (h + 1),
                                                    (kb % 4) * P:(kb % 4 + 1) * P],
                            rhs=qkv_sb[qc][0][D * h:D * (h + 1), lqsl],
                            start=True, stop=True)
                    nc.scalar.activation(
                        et[:, :, :N],
                        lp[:, :].rearrange("p (b n) -> p b n", b=2)[:, :, :N],
                        EXP)
                if diag:
                    nc.vector.tensor_mul(
                        et[:, :, 0:P], et[:, :, 0:P],
                        tri[:, None, :].to_broadcast((P, 2, P)))
                if carry:
                    carry.pop(0)()
                    if carry:
                        carry.pop(0)()
                elif len(pend) >= 3:
                    flush(pend.pop(0), False)
                pend.append((kb, et, qoff, N))
                if fillers and kb % 2 == 1:
                    fillers.pop(0)()
            for c in carry:
                c()
            for f in fillers:
                f()

            res = {"le": last_exp}

            def finalize():
                # normalize rows 0:64 by the exp sums replicated in rows
                # 64:128, both heads into one staging tile, then scatter
                # stripes into the a2a input buffer.
                stage = stagep.tile([P, QC], BF16, tag="stage", name="stage")
                for h in range(HPC):
                    rb = smallp.tile([D, QC], F32, tag="rb", name="rb")
                    sm = smallp.tile([D, QC], F32, tag="sm", name="sm")
                    if last:
                        nc.vector.tensor_copy(
                            sm[:, :], avt[D:2 * D, h * QC:(h + 1) * QC])
                        nc.vector.reciprocal_approx_fast(rb[:, :], sm[:, :])
                        nc.vector.tensor_mul(
                            stage[D * h:D * (h + 1), :],
                            avt[0:D, h * QC:(h + 1) * QC], rb[:, :])
                    else:
                        nc.vector.tensor_copy(sm[:, :], avp[h][D:2 * D, :])
                        nc.vector.reciprocal_approx_fast(rb[:, :], sm[:, :])
                        nc.vector.tensor_mul(
                            stage[D * h:D * (h + 1), :],
                            avp[h][0:D, :], rb[:, :])
                g, slot = GSLOT[qc]
                sdmas = []
                for h in range(HPC):
                    sdmas.append(nc.sync.dma_start(
                        a2a_in[g][:, :].rearrange("(j r) q -> r j q", r=P)
                        [D * h:D * (h + 1), :, slot * ST:(slot + 1) * ST],
                        stage[D * h:D * (h + 1), :]
                        .rearrange("p (j q) -> p j q", q=ST)))
                res["stage"] = stage
                res["sdmas"] = sdmas

            if defer:
                cl = []
                n = len(pend)
                for i in range(n):
                    item = pend[i]
                    cl.append(lambda item=item, lastf=(i == n - 1):
                              res.__setitem__("last_av", flush(item, lastf)))
                cl.append(finalize)
                return cl, res
            while pend:
                res["last_av"] = flush(pend.pop(0), not pend)
            finalize()
            return None, res

        def fire_a2a(g, stage_dmas):
            # explicit doorbell with hard sync deps on every stage write:
            # Tile's automatic single-sem wait has been observed to gate the
            # doorbell on the wrong queue position.
            cc = nc.gpsimd.collective_compute(
                "AllToAll", mybir.AluOpType.bypass,
                replica_groups=[list(range(NCORES))],
                ins=[a2a_in[g][:, :].opt()],
                outs=[a2a_out[g][:, :].opt()])
            for sd in stage_dmas:
                add_dep_helper(cc.ins, sd.ins, sync=True,
                               reason="a2a after all stage writes")
            return cc

        def emit_proj_pair(at_sl, rowbase, anchor, out_eng, out_pin=None):
            # projection for one stripe pair: 128 output rows, full-width
            # [128,128] lhsT via col-tiled half matmuls (the pair runs
            # concurrently on the PE).
            # anchor: keep these instructions behind the attention stream in
            # the static schedule. out_pin: keep the out DMAs behind the
            # given instruction on their queue (the g1 doorbell's gating
            # relies on in-order queue completion of the stage writes).
            def pin(inst):
                if anchor is not None:
                    add_dep_helper(inst.ins, anchor.ins, sync=False,
                                   reason="proj after attention")
                return inst
            last_mm = None
            ob = outp.tile([P, E], F32, tag="ob", name="ob")
            for n in range(E // QC):
                nsl = slice(n * QC, (n + 1) * QC)
                pp = psmm.tile([P, QC], F32, tag="mmp", name="proj_ps")
                for k in range(KCH):
                    for half in range(2):
                        pin(nc.tensor.matmul(
                            pp[half * ST:(half + 1) * ST, :],
                            lhsT=at_sl(k)[:, half * ST:(half + 1) * ST],
                            rhs=wp_sb[:, k, nsl],
                            start=(k == 0), stop=False,
                            tile_position=(0, half * ST)))
                last_mm = pin(nc.tensor.matmul(
                    pp[:, :], lhsT=ones1[:, :], rhs=bp_sb[:, nsl],
                    start=False, stop=True))
                nc.vector.tensor_copy(ob[:, nsl], pp[:, :])
                d = out_eng[n].dma_start(out[rowbase:rowbase + P, nsl],
                                         ob[:, nsl])
                if out_pin is not None:
                    add_dep_helper(d.ins, out_pin.ins, sync=False,
                                   reason="out writes after stage writes")
            return last_mm

        # ---- emission: interleave qkv chunks with attention so both PE
        # phases and the ACT exp stream overlap; attention runs 1,2,3,0 so
        # the final collective only waits on the shortest chunk (qc0).
        k0 = emit_qkv_m(0, 1)
        emit_qkv_m(0, 2)
        for kb in range(0, 4):
            emit_vtrans(kb)
        emit_qkv_m(1, 0)
        # wpt on the (otherwise idle) gpsimd SWDGE queue, gated on the first
        # qkv output so its 2MB doesn't steal prologue HBM bandwidth
        for k in range(KCH):
            d = nc.gpsimd.dma_start(wp_sb[:, k, :], wpt[k * P:(k + 1) * P, :])
            add_dep_helper(d.ins, k0.ins, sync=True,
                           reason="wpt after first qkv")
        nc.gpsimd.dma_start(bp_sb[:, :], bp[:, :])
        f1 = [lambda: emit_qkv_m(1, 1), lambda: emit_qkv_m(1, 2)] + \
             [lambda kb=kb: emit_vtrans(kb) for kb in range(4, 8)] + \
             [lambda: emit_qkv_m(0, 0)] + \
             [lambda m=m: emit_qkv_m(2, m) for m in range(3)] + \
             [lambda kb=kb: emit_vtrans(kb) for kb in range(8, 12)]
        c1, r1 = emit_attn(1, f1, defer=True)
        f2 = [lambda m=m: emit_qkv_m(3, m) for m in range(3)] + \
             [lambda kb=kb: emit_vtrans(kb) for kb in range(12, 16)]
        c2, r2 = emit_attn(2, f2, carry=c1, defer=True)
        c3, r3 = emit_attn(3, carry=c2, defer=True)
        fire_a2a(0, r1["sdmas"] + r2["sdmas"])
        _, r0 = emit_attn(0, last=True, carry=c3)
        fire_a2a(1, r3["sdmas"] + r0["sdmas"])
        last_av, stage0, le0 = r0["last_av"], r0["stage"], r0["le"]
        sd0 = r0["sdmas"][-1]
        # at12 batched load on scalar (idle once chunk-0's exps are done),
        # pinned behind the last exp so it can't head-of-line block them;
        # fires as soon as the g0 collective completes.
        d12 = nc.scalar.dma_start(
            at12[:, :, :],
            a2a_out[0][:, :].rearrange("(k p) c -> p k c", p=P))
        add_dep_helper(d12.ins, le0.ins, sync=False,
                       reason="at12 load after chunk-0 exps")
        # proj pair (1,2): runs on the PE right after the last AV, covering
        # the first part of the g1 collective window. Its out DMAs sit on
        # sync BEHIND the chunk-0 stage writes (doorbell gating relies on
        # in-order completion of that queue).
        p1_mm = emit_proj_pair(lambda k: at12[:, k, :], 0, last_av,
                               [nc.sync, nc.sync], out_pin=sd0)
        # keep-warm chain: a marker copy on vector (right after chunk-0's
        # normalize) gates a scalar-engine copy chain; a tiny matmul after
        # each link keeps the HAM activity window non-idle through the
        # collective wait so proj2 runs warm.
        nc.vector.tensor_copy(wa[1][0:1, 0:16], stage0[0:1, 0:16])
        prev_mm = p1_mm
        last_cp = None
        for i in range(10):
            src, dst = wa[i % 2], wa[(i + 1) % 2]
            cp = nc.scalar.copy(dst[:, 0:1024], src[:, 0:1024])
            if last_cp is None:
                add_dep_helper(cp.ins, d12.ins, sync=False,
                               reason="warm copies after at12 trigger")
            last_cp = cp
            wps = psmm.tile([P, QC], F32, tag="mmp", name="keepwarm_ps")
            m = nc.tensor.matmul(wps[:, 0:P], lhsT=dst[:, 0:P],
                                 rhs=dst[:, 0:P], start=True, stop=True)
            add_dep_helper(m.ins, prev_mm.ins, sync=False,
                           reason="keepwarm after proj1")
            prev_mm = m
        # gathered loads for pair (3,0): four quarter loads on the scalar +
        # sync queues (the sync pair sits behind the proj1 out writes; all
        # of them only fire once the g1 collective lands anyway).
        at30_d = []
        for i, eng in enumerate((nc.scalar, nc.sync, nc.scalar, nc.sync)):
            d = eng.dma_start(
                at30[i][:, :, :],
                a2a_out[1][i * (KCH // 4) * P:(i + 1) * (KCH // 4) * P, :]
                .rearrange("(k p) c -> p k c", p=P))
            add_dep_helper(d.ins, last_cp.ins if eng is nc.scalar else sd0.ins,
                           sync=False, reason="at30 loads late in queue")
            at30_d.append(d)
        emit_proj_pair(lambda k: at30[k // (KCH // 4)][:, k % (KCH // 4), :],
                       P, prev_mm, [nc.sync, nc.scalar], out_pin=sd0)


_NC_CACHE = None


def _get_nc():
    global _NC_CACHE
    if _NC_CACHE is None:
        _NC_CACHE = build_nc()
    return _NC_CACHE


def _f8(a):
    return np.clip(a, -240, 240).astype(ml_dtypes.float8_e4m3fn)


def make_in_maps(x, w_attn, b_attn, w_proj, b_proj):
    bf16 = ml_dtypes.bfloat16
    x = np.asarray(x, dtype=np.float32)
    w_attn = np.asarray(w_attn, dtype=np.float32)
    b_attn = np.asarray(b_attn, dtype=np.float32)
    w_proj = np.asarray(w_proj, dtype=np.float32)
    b_proj = np.asarray(b_proj, dtype=np.float32)

    xt = _f8(np.ascontiguousarray(x.T) * SX)             # (E, S) fp8
    wpt = np.ascontiguousarray(w_proj.T).astype(bf16)    # (E, E)
    bpa = np.ascontiguousarray(b_proj[None, :]).astype(bf16)
    scale = 1.0 / np.sqrt(D)

    in_maps = []
    for c in range(NCORES):
        rq = slice(F * c, F * (c + 1))
        rk = slice(E + F * c, E + F * (c + 1))
        rv = slice(2 * E + F * c, 2 * E + F * (c + 1))
        wqkv = np.ascontiguousarray(np.concatenate(
            [w_attn[rq] * scale, w_attn[rk], w_attn[rv]], axis=0).T)  # (E, 3F)
        # swizzle to [p, (m g j c)]: DoubleRow pairs (g, j) of contraction
        # row-blocks, one wide-line DMA on device
        wq_sw = np.ascontiguousarray(
            wqkv.reshape(KCH // 2, 2, P, 3, P).transpose(2, 3, 0, 1, 4)
            .reshape(P, 3 * KCH * P))
        bq = np.stack([b_attn[rq] * scale, b_attn[rk], b_attn[rv]], axis=1)
        in_maps.append({
            "xt": xt,
            "wqkv": _f8(wq_sw * SW),
            "bqkv": np.ascontiguousarray(bq, dtype=np.float32),
            "wpt": wpt,
            "bp": bpa,
        })
    return in_maps


def run(inputs, trace=False, **kw):
    from concourse.bass_utils import run_bass_kernel_spmd
    nc = _get_nc()
    in_maps = make_in_maps(**inputs)
    res = run_bass_kernel_spmd(nc, in_maps, core_ids=list(range(NCORES)),
                               trace=trace, **kw)
    # core j's out row blocks are stripes for qc = 1,2,3,0 in that order;
    # stripe qc covers global rows 512*qc + 64*j .. +64
    full = np.empty((S, E), dtype=np.float32)
    for j in range(NCORES):
        o = res.results[j]["out"]                        # (256, E)
        for blk, qc in enumerate([1, 2, 3, 0]):
            full[QC * qc + ST * j: QC * qc + ST * (j + 1), :] = \
                o[ST * blk: ST * (blk + 1), :]
    return full, res


def kernel(**inputs):
    full, _ = run(inputs, trace=False)
    return full
